# revision 9
# baseline (speedup 1.0000x reference)
"""Llama4-style MoE (top-1 routing, 32 experts + shared expert) on 8 Trainium2
NeuronCores.

Sharding strategy (expert-parallel, per the spec hint):
  - The top-1 router + token dispatch IS the input sharding: the host computes
    logits/argmax/sigmoid (0.25% of the module FLOPs), sorts tokens by expert,
    and hands each core the scaled+transposed token block for its 4 experts.
  - Routed expert weights are sharded over the expert axis (4 experts/core).
  - The shared-expert SwiGLU is token-parallel: core c takes tokens
    [c*1024, (c+1)*1024).
  - All 8 cores run ONE SPMD program: segment capacities are identical across
    cores (experts are assigned to (core, slot) by descending token count so
    slot s has capacity = max count within its group of 8 experts); which
    expert's weights/tokens live in a slot differs per core via the inputs.
  - Combine: routed rows are scattered back to token order on the host and
    added to the shared output (disjoint row writes + one add).

Device schedule (v2): the shared expert's three stages (gate/up icg0,
gate/up icg1, down) are interleaved between the routed slots so weight
DMA demand stays under the HBM roofline the whole time, and both down
projections are computed transposed (outT = Wd.T @ h with tokens as the
matmul free dim) so capacity remainders don't pay a full 128-row PSUM
pass. Weight/activation DMAs at the head are ordered chunk-by-chunk so
the PE starts within a few microseconds.

Device kernel: big GEMMs in bf16 (weights cast on host), fp32 PSUM
accumulation, silu on the scalar engine, outputs fp32.
"""

import numpy as np
import ml_dtypes

import concourse.bass as bass
import concourse.mybir as mybir
import concourse.tile as tile
from concourse import bacc
from concourse.bass_utils import run_bass_kernel_spmd

H, I, E = 1024, 2048, 32
B, S = 4, 2048
T = B * S
NCORES = 8
EPC = E // NCORES  # experts per core
HC = H // 128      # 8 contraction chunks for gate/up
IC = I // 128      # 16 contraction chunks for down
HB = H // 128      # 8 output-row blocks for the transposed down
TSH = T // NCORES  # shared-expert tokens per core

BF16 = mybir.dt.bfloat16
FP32 = mybir.dt.float32
BF16_NP = ml_dtypes.bfloat16

TRACE = False
LAST_RESULTS = None

_PROGRAM_CACHE = {}
_PREP_CACHE = {}


def _ceil_div(a, b):
    return (a + b - 1) // b


def _emit_gateup(nc, pools, acts_sb, acts_off, W, wg_ap, wu_ap, icg, ht_sb,
                 wg_cb=None, wu_cb=None):
    """Half the SwiGLU front: for intermediate half `icg`, compute
    ht_sb[:, icg*8+ic8, 0:W] = silu(a@Wg) * (a@Wu) for the [H, W] token
    block a = acts_sb[:, :, acts_off:acts_off+W]. Weights stream in 512-col
    half-granules so lookahead stays fine-grained. wg_cb/wu_cb(hc) hooks let
    the head interleave activation DMAs with the first weight chunks."""
    wgu, sgp, psum = pools["wgu"], pools["sg"], pools["psum"]
    TCH = _ceil_div(W, 512)

    for half in range(2):
        c0 = icg * 1024 + half * 512
        wg_sb = wgu.tile([128, HC, 512], BF16, tag="wg")
        wu_sb = wgu.tile([128, HC, 512], BF16, tag="wu")
        for hc in range(HC):
            nc.sync.dma_start(
                wg_sb[:, hc, :], wg_ap[hc * 128:(hc + 1) * 128, c0:c0 + 512]
            )
            if half == 0 and wg_cb is not None:
                wg_cb(hc)
        for hc in range(HC):
            nc.sync.dma_start(
                wu_sb[:, hc, :], wu_ap[hc * 128:(hc + 1) * 128, c0:c0 + 512]
            )
        if half == 0 and wu_cb is not None:
            wu_cb()
        for tch in range(TCH):
            o = tch * 512
            w = min(512, W - o)
            for ic4 in range(4):
                ic = icg * 8 + half * 4 + ic4
                pg = psum.tile([128, 512], FP32, tag="pg")
                for hc in range(HC):
                    nc.tensor.matmul(
                        pg[:, :w],
                        lhsT=wg_sb[:, hc, ic4 * 128:(ic4 + 1) * 128],
                        rhs=acts_sb[:, hc, acts_off + o:acts_off + o + w],
                        start=(hc == 0), stop=(hc == HC - 1),
                    )
                sg = sgp.tile([128, 512], BF16, tag="sg")
                nc.scalar.activation(
                    sg[:, :w], pg[:, :w], mybir.ActivationFunctionType.Silu
                )
                pu = psum.tile([128, 512], FP32, tag="pu")
                for hc in range(HC):
                    nc.tensor.matmul(
                        pu[:, :w],
                        lhsT=wu_sb[:, hc, ic4 * 128:(ic4 + 1) * 128],
                        rhs=acts_sb[:, hc, acts_off + o:acts_off + o + w],
                        start=(hc == 0), stop=(hc == HC - 1),
                    )
                nc.vector.tensor_tensor(
                    ht_sb[:, ic, o:o + w], sg[:, :w], pu[:, :w],
                    op=mybir.AluOpType.mult,
                )


def _emit_down(nc, pools, ht_sb, W, wd_ap, out_ap, out_col0):
    """Transposed down projection: outT[H, W] = Wd.T @ ht (tokens stay the
    free dim, so odd capacities don't pay a full 128-row pass). Weights
    stream in 512-col half-granules."""
    wdp, odp, psum = pools["wd"], pools["od"], pools["psum"]
    TCH = _ceil_div(W, 512)
    for half in range(2):
        c0 = half * 512
        wd_sb = wdp.tile([128, IC, 512], BF16, tag="wd")
        for ic in range(IC):
            nc.sync.dma_start(
                wd_sb[:, ic, :], wd_ap[ic * 128:(ic + 1) * 128, c0:c0 + 512]
            )
        for tch in range(TCH):
            o = tch * 512
            w = min(512, W - o)
            for hb4 in range(4):
                hb = half * 4 + hb4
                pd = psum.tile([128, 512], FP32, tag="pd")
                for ic in range(IC):
                    nc.tensor.matmul(
                        pd[:, :w],
                        lhsT=wd_sb[:, ic, hb4 * 128:(hb4 + 1) * 128],
                        rhs=ht_sb[:, ic, o:o + w],
                        start=(ic == 0), stop=(ic == IC - 1),
                    )
                od = odp.tile([128, 512], FP32, tag="od")
                nc.vector.tensor_copy(od[:, :w], pd[:, :w])
                nc.sync.dma_start(
                    out_ap[hb, :, out_col0 + o:out_col0 + o + w], od[:, :w]
                )


def _build_program(caps):
    nloc = sum(caps)
    nc = bacc.Bacc("TRN2", target_bir_lowering=False, debug=False,
                   num_devices=NCORES)

    xsT = nc.dram_tensor("xsT", [HC, 128, nloc], BF16, kind="ExternalInput")
    xshT = nc.dram_tensor("xshT", [HC, 128, TSH], BF16, kind="ExternalInput")
    rgw = nc.dram_tensor("rgw", [EPC, H, I], BF16, kind="ExternalInput")
    ruw = nc.dram_tensor("ruw", [EPC, H, I], BF16, kind="ExternalInput")
    rdw = nc.dram_tensor("rdw", [EPC, I, H], BF16, kind="ExternalInput")
    sgw = nc.dram_tensor("sgw", [H, I], BF16, kind="ExternalInput")
    suw = nc.dram_tensor("suw", [H, I], BF16, kind="ExternalInput")
    sdw = nc.dram_tensor("sdw", [I, H], BF16, kind="ExternalInput")
    routT = nc.dram_tensor("routT", [HB, 128, nloc], FP32,
                           kind="ExternalOutput")
    shoutT = nc.dram_tensor("shoutT", [HB, 128, TSH], FP32,
                            kind="ExternalOutput")

    maxcap = max(caps)

    with tile.TileContext(nc) as tc:
        with (
            tc.tile_pool(name="acts", bufs=1) as acts,
            tc.tile_pool(name="wgu", bufs=3) as wgu,
            tc.tile_pool(name="wd", bufs=3) as wdp,
            tc.tile_pool(name="rht", bufs=1) as rhtp,
            tc.tile_pool(name="sg", bufs=3) as sgp,
            tc.tile_pool(name="od", bufs=4) as odp,
            tc.tile_pool(name="psum", bufs=2, space="PSUM") as psum,
        ):
            pools = {"wgu": wgu, "wd": wdp, "sg": sgp, "od": odp, "psum": psum}

            xsT_sb = acts.tile([128, HC, nloc], BF16)
            xshT_sb = acts.tile([128, HC, TSH], BF16)
            sh_ht = acts.tile([128, IC, TSH], BF16)

            offs = [sum(caps[:s]) for s in range(EPC)]

            # Head DMA order decides when the PE starts: interleave the first
            # shared-gate weight chunks with the first 512 token columns of
            # the shared activations (~2 MB before the first matmul), then the
            # up weights, then the remaining activations stream in behind.
            def head_wg_cb(hc):
                nc.sync.dma_start(xshT_sb[:, hc, 0:512], xshT[hc, :, 0:512])

            def head_wu_cb():
                for hc in range(HC):
                    nc.sync.dma_start(xshT_sb[:, hc, 512:TSH],
                                      xshT[hc, :, 512:TSH])
                off = 0
                for s in range(EPC):
                    for hc in range(HC):
                        nc.sync.dma_start(
                            xsT_sb[:, hc, off:off + caps[s]],
                            xsT[hc, :, off:off + caps[s]],
                        )
                    off += caps[s]

            # Interleave the shared expert's stages between routed slots so
            # weight-DMA demand stays smooth: shared stages are compute-dense
            # (one 4 MB weight load feeds 55 us of PE work), giving the next
            # routed slot's 12 MB time to stream in.
            def routed(s):
                r_ht = rhtp.tile([128, IC, maxcap], BF16, tag="rht")
                _emit_gateup(nc, pools, xsT_sb, offs[s], caps[s],
                             rgw[s], ruw[s], 0, r_ht)
                _emit_gateup(nc, pools, xsT_sb, offs[s], caps[s],
                             rgw[s], ruw[s], 1, r_ht)
                _emit_down(nc, pools, r_ht, caps[s], rdw[s], routT, offs[s])

            _emit_gateup(nc, pools, xshT_sb, 0, TSH, sgw, suw, 0, sh_ht,
                         wg_cb=head_wg_cb, wu_cb=head_wu_cb)
            routed(0)
            _emit_gateup(nc, pools, xshT_sb, 0, TSH, sgw, suw, 1, sh_ht)
            routed(1)
            _emit_down(nc, pools, sh_ht, TSH, sdw, shoutT, 0)
            routed(2)
            routed(3)
    nc.finalize()
    return nc


def _get_program(caps):
    caps = tuple(caps)
    if caps not in _PROGRAM_CACHE:
        _PROGRAM_CACHE[caps] = _build_program(caps)
    return _PROGRAM_CACHE[caps]


def _to_bf16(a):
    return np.ascontiguousarray(a).astype(BF16_NP)


def kernel(**inputs):
    global LAST_RESULTS
    x = np.ascontiguousarray(
        np.asarray(inputs["hidden_states"], dtype=np.float32)
    ).reshape(T, H)
    gate_w = np.asarray(inputs["gate_w"], dtype=np.float32)

    # ---- router (host; this decides the sharding) ----
    logits = x @ gate_w
    ids = logits.argmax(-1)
    topv = logits.max(-1)
    scores = (1.0 / (1.0 + np.exp(-topv.astype(np.float64)))).astype(np.float32)

    counts = np.bincount(ids, minlength=E)
    order = np.argsort(-counts, kind="stable")
    caps = tuple(int(counts[order[s * NCORES]]) for s in range(EPC))
    caps = tuple(max(c, 128) for c in caps)
    nloc = sum(caps)
    nc = _get_program(caps)

    tok = [np.where(ids == e)[0] for e in range(E)]

    # ---- per-core inputs ----
    key = tuple(id(inputs[k]) for k in
                ("routed_gate_w", "routed_up_w", "routed_down_w",
                 "shared_gate_w", "shared_up_w", "shared_down_w"))
    wcache = _PREP_CACHE.get("wkey") == key
    if not wcache:
        _PREP_CACHE.clear()
        _PREP_CACHE["wkey"] = key
        _PREP_CACHE["rgw"] = _to_bf16(np.asarray(inputs["routed_gate_w"], np.float32))
        _PREP_CACHE["ruw"] = _to_bf16(np.asarray(inputs["routed_up_w"], np.float32))
        _PREP_CACHE["rdw"] = _to_bf16(np.asarray(inputs["routed_down_w"], np.float32))
        _PREP_CACHE["sgw"] = _to_bf16(np.asarray(inputs["shared_gate_w"], np.float32))
        _PREP_CACHE["suw"] = _to_bf16(np.asarray(inputs["shared_up_w"], np.float32))
        _PREP_CACHE["sdw"] = _to_bf16(np.asarray(inputs["shared_down_w"], np.float32))
    rgw_all, ruw_all, rdw_all = (_PREP_CACHE["rgw"], _PREP_CACHE["ruw"],
                                 _PREP_CACHE["rdw"])
    sgw, suw, sdw = _PREP_CACHE["sgw"], _PREP_CACHE["suw"], _PREP_CACHE["sdw"]

    in_maps = []
    core_segs = []
    for c in range(NCORES):
        segs = [int(order[s * NCORES + c]) for s in range(EPC)]
        core_segs.append(segs)
        xs_loc = np.zeros((nloc, H), np.float32)
        off = 0
        for s, e in enumerate(segs):
            tl = tok[e]
            xs_loc[off:off + len(tl)] = x[tl] * scores[tl][:, None]
            off += caps[s]
        xsT_np = np.ascontiguousarray(xs_loc.T.reshape(HC, 128, nloc)).astype(BF16_NP)
        xshT_np = np.ascontiguousarray(
            x[c * TSH:(c + 1) * TSH].T.reshape(HC, 128, TSH)
        ).astype(BF16_NP)
        in_maps.append({
            "xsT": xsT_np,
            "xshT": xshT_np,
            "rgw": np.ascontiguousarray(rgw_all[segs]),
            "ruw": np.ascontiguousarray(ruw_all[segs]),
            "rdw": np.ascontiguousarray(rdw_all[segs]),
            "sgw": sgw, "suw": suw, "sdw": sdw,
        })

    res = run_bass_kernel_spmd(nc, in_maps, core_ids=list(range(NCORES)),
                               trace=TRACE)
    LAST_RESULTS = res

    # ---- combine ----
    out = np.zeros((T, H), np.float32)
    for c in range(NCORES):
        routT = res.results[c]["routT"].reshape(H, nloc)
        off = 0
        for s, e in enumerate(core_segs[c]):
            tl = tok[e]
            out[tl] = routT[:, off:off + len(tl)].T
            off += caps[s]
    for c in range(NCORES):
        shoutT = res.results[c]["shoutT"].reshape(H, TSH)
        out[c * TSH:(c + 1) * TSH] += shoutT.T
    return out.reshape(B, S, H)


# revision 14
# speedup vs baseline: 1.1184x; 1.1184x over previous
"""Llama4-style MoE (top-1 routing, 32 experts + shared expert) on 8 Trainium2
NeuronCores.

Sharding strategy (expert-parallel, per the spec hint):
  - The top-1 router + token dispatch IS the input sharding: the host computes
    logits/argmax/sigmoid (0.25% of the module FLOPs), sorts tokens by expert,
    and hands each core the scaled+transposed token block for its 4 experts.
  - Routed expert weights are sharded over the expert axis (4 experts/core).
  - The shared-expert SwiGLU is token-parallel: core c takes tokens
    [c*1024, (c+1)*1024).
  - All 8 cores run ONE SPMD program: segment capacities are identical across
    cores (experts are assigned to (core, slot) by descending token count so
    slot s has capacity = max count within its group of 8 experts); which
    expert's weights/tokens live in a slot differs per core via the inputs.
  - Combine: routed rows are scattered back to token order on the host and
    added to the shared output (disjoint row writes + one add).

Device schedule (v2): the shared expert's three stages (gate/up icg0,
gate/up icg1, down) are interleaved between the routed slots so weight
DMA demand stays under the HBM roofline the whole time, and both down
projections are computed transposed (outT = Wd.T @ h with tokens as the
matmul free dim) so capacity remainders don't pay a full 128-row PSUM
pass. Weight/activation DMAs at the head are ordered chunk-by-chunk so
the PE starts within a few microseconds.

Device kernel: big GEMMs in bf16 (weights cast on host), fp32 PSUM
accumulation, silu on the scalar engine, outputs fp32.
"""

import numpy as np
import ml_dtypes

import concourse.bass as bass
import concourse.mybir as mybir
import concourse.tile as tile
from concourse import bacc
from concourse.bass_utils import run_bass_kernel_spmd

H, I, E = 1024, 2048, 32
B, S = 4, 2048
T = B * S
NCORES = 8
EPC = E // NCORES  # experts per core
HC = H // 128      # 8 contraction chunks for gate/up
IC = I // 128      # 16 contraction chunks for down
HB = H // 128      # 8 output-row blocks for the transposed down
TSH = T // NCORES  # shared-expert tokens per core

BF16 = mybir.dt.bfloat16
FP32 = mybir.dt.float32
BF16_NP = ml_dtypes.bfloat16

TRACE = False
LAST_RESULTS = None

_PROGRAM_CACHE = {}
_PREP_CACHE = {}


def _ceil_div(a, b):
    return (a + b - 1) // b


def _emit_gateup(nc, pools, acts_sb, acts_off, W, wg_ap, wu_ap, icg, ht_sb,
                 wg_cb=None, wu_cb=None):
    """Half the SwiGLU front: for intermediate half `icg`, compute
    ht_sb[:, icg*8+ic8, 0:W] = silu(a@Wg) * (a@Wu) for the [H, W] token
    block a = acts_sb[:, :, acts_off:acts_off+W]. Weights stream in 512-col
    half-granules so lookahead stays fine-grained. wg_cb/wu_cb(hc) hooks let
    the head interleave activation DMAs with the first weight chunks."""
    wgu, sgp, psum = pools["wgu"], pools["sg"], pools["psum"]
    TCH = _ceil_div(W, 512)

    wg_sb = wgu.tile([128, HC, 1024], BF16, tag="wg")
    wu_sb = wgu.tile([128, HC, 1024], BF16, tag="wu")
    for hc in range(HC):
        nc.sync.dma_start(
            wg_sb[:, hc, :],
            wg_ap[hc * 128:(hc + 1) * 128, icg * 1024:(icg + 1) * 1024],
        )
        if wg_cb is not None:
            wg_cb(hc)
    for hc in range(HC):
        nc.sync.dma_start(
            wu_sb[:, hc, :],
            wu_ap[hc * 128:(hc + 1) * 128, icg * 1024:(icg + 1) * 1024],
        )
    if wu_cb is not None:
        wu_cb()
    for tch in range(TCH):
        o = tch * 512
        w = min(512, W - o)
        for ic8 in range(8):
            ic = icg * 8 + ic8
            pg = psum.tile([128, 512], FP32, tag="pg")
            for hc in range(HC):
                nc.tensor.matmul(
                    pg[:, :w],
                    lhsT=wg_sb[:, hc, ic8 * 128:(ic8 + 1) * 128],
                    rhs=acts_sb[:, hc, acts_off + o:acts_off + o + w],
                    start=(hc == 0), stop=(hc == HC - 1),
                )
            sg = sgp.tile([128, 512], BF16, tag="sg")
            nc.scalar.activation(
                sg[:, :w], pg[:, :w], mybir.ActivationFunctionType.Silu
            )
            pu = psum.tile([128, 512], FP32, tag="pu")
            for hc in range(HC):
                nc.tensor.matmul(
                    pu[:, :w],
                    lhsT=wu_sb[:, hc, ic8 * 128:(ic8 + 1) * 128],
                    rhs=acts_sb[:, hc, acts_off + o:acts_off + o + w],
                    start=(hc == 0), stop=(hc == HC - 1),
                )
            nc.vector.tensor_tensor(
                ht_sb[:, ic, o:o + w], sg[:, :w], pu[:, :w],
                op=mybir.AluOpType.mult,
            )


def _emit_down(nc, pools, ht_sb, W, wd_ap, out_ap, out_col0):
    """Transposed down projection: outT[H, W] = Wd.T @ ht (tokens stay the
    free dim, so odd capacities don't pay a full 128-row pass). Weights
    stream in 512-col half-granules."""
    wdp, odp, psum = pools["wd"], pools["od"], pools["psum"]
    TCH = _ceil_div(W, 512)
    wd_sb = wdp.tile([128, IC, 1024], BF16, tag="wd")
    for ic in range(IC):
        nc.sync.dma_start(wd_sb[:, ic, :], wd_ap[ic * 128:(ic + 1) * 128, :])
    for tch in range(TCH):
        o = tch * 512
        w = min(512, W - o)
        for hb in range(HB):
            pd = psum.tile([128, 512], FP32, tag="pd")
            for ic in range(IC):
                nc.tensor.matmul(
                    pd[:, :w],
                    lhsT=wd_sb[:, ic, hb * 128:(hb + 1) * 128],
                    rhs=ht_sb[:, ic, o:o + w],
                    start=(ic == 0), stop=(ic == IC - 1),
                )
            od = odp.tile([128, 512], FP32, tag="od")
            nc.vector.tensor_copy(od[:, :w], pd[:, :w])
            nc.sync.dma_start(
                out_ap[hb, :, out_col0 + o:out_col0 + o + w], od[:, :w]
            )


def _build_program(caps):
    nloc = sum(caps)
    nc = bacc.Bacc("TRN2", target_bir_lowering=False, debug=False,
                   num_devices=NCORES)

    xsT = nc.dram_tensor("xsT", [HC, 128, nloc], BF16, kind="ExternalInput")
    xshT = nc.dram_tensor("xshT", [HC, 128, TSH], BF16, kind="ExternalInput")
    rgw = nc.dram_tensor("rgw", [EPC, H, I], BF16, kind="ExternalInput")
    ruw = nc.dram_tensor("ruw", [EPC, H, I], BF16, kind="ExternalInput")
    rdw = nc.dram_tensor("rdw", [EPC, I, H], BF16, kind="ExternalInput")
    sgw = nc.dram_tensor("sgw", [H, I], BF16, kind="ExternalInput")
    suw = nc.dram_tensor("suw", [H, I], BF16, kind="ExternalInput")
    sdw = nc.dram_tensor("sdw", [I, H], BF16, kind="ExternalInput")
    routT = nc.dram_tensor("routT", [HB, 128, nloc], FP32,
                           kind="ExternalOutput")
    shoutT = nc.dram_tensor("shoutT", [HB, 128, TSH], FP32,
                            kind="ExternalOutput")

    maxcap = max(caps)

    with tile.TileContext(nc) as tc:
        with (
            tc.tile_pool(name="acts", bufs=1) as acts,
            tc.tile_pool(name="xs", bufs=2) as xsp,
            tc.tile_pool(name="wgu", bufs=2) as wgu,
            tc.tile_pool(name="wd", bufs=2) as wdp,
            tc.tile_pool(name="rht", bufs=1) as rhtp,
            tc.tile_pool(name="sg", bufs=2) as sgp,
            tc.tile_pool(name="od", bufs=2) as odp,
            tc.tile_pool(name="psum", bufs=2, space="PSUM") as psum,
        ):
            pools = {"wgu": wgu, "wd": wdp, "sg": sgp, "od": odp, "psum": psum}

            xshT_sb = acts.tile([128, HC, TSH], BF16)
            sh_ht = acts.tile([128, IC, TSH], BF16)

            offs = [sum(caps[:s]) for s in range(EPC)]

            # Head DMA order decides when the PE starts: interleave the first
            # shared-gate weight chunks with the first 512 token columns of
            # the shared activations (~2 MB before the first matmul), then the
            # up weights, then the rest of the shared activations.
            def head_wg_cb(hc):
                nc.sync.dma_start(xshT_sb[:, hc, 0:512], xshT[hc, :, 0:512])

            def head_wu_cb():
                for hc in range(HC):
                    nc.sync.dma_start(xshT_sb[:, hc, 512:TSH],
                                      xshT[hc, :, 512:TSH])

            # Interleave the shared expert's stages between routed slots so
            # weight-DMA demand stays smooth: shared stages are compute-dense
            # (one 4 MB weight load feeds 55 us of PE work), giving the next
            # routed slot's 12 MB time to stream in.
            def routed(s):
                xs_sb = xsp.tile([128, HC, maxcap], BF16, tag="xs")
                for hc in range(HC):
                    nc.sync.dma_start(
                        xs_sb[:, hc, 0:caps[s]],
                        xsT[hc, :, offs[s]:offs[s] + caps[s]],
                    )
                r_ht = rhtp.tile([128, IC, maxcap], BF16, tag="rht")
                _emit_gateup(nc, pools, xs_sb, 0, caps[s],
                             rgw[s], ruw[s], 0, r_ht)
                _emit_gateup(nc, pools, xs_sb, 0, caps[s],
                             rgw[s], ruw[s], 1, r_ht)
                _emit_down(nc, pools, r_ht, caps[s], rdw[s], routT, offs[s])

            _emit_gateup(nc, pools, xshT_sb, 0, TSH, sgw, suw, 0, sh_ht,
                         wg_cb=head_wg_cb, wu_cb=head_wu_cb)
            routed(0)
            _emit_gateup(nc, pools, xshT_sb, 0, TSH, sgw, suw, 1, sh_ht)
            routed(1)
            _emit_down(nc, pools, sh_ht, TSH, sdw, shoutT, 0)
            routed(2)
            routed(3)
    nc.finalize()
    return nc


def _get_program(caps):
    caps = tuple(caps)
    if caps not in _PROGRAM_CACHE:
        _PROGRAM_CACHE[caps] = _build_program(caps)
    return _PROGRAM_CACHE[caps]


def _to_bf16(a):
    return np.ascontiguousarray(a).astype(BF16_NP)


def kernel(**inputs):
    global LAST_RESULTS
    x = np.ascontiguousarray(
        np.asarray(inputs["hidden_states"], dtype=np.float32)
    ).reshape(T, H)
    gate_w = np.asarray(inputs["gate_w"], dtype=np.float32)

    # ---- router (host; this decides the sharding) ----
    logits = x @ gate_w
    ids = logits.argmax(-1)
    topv = logits.max(-1)
    scores = (1.0 / (1.0 + np.exp(-topv.astype(np.float64)))).astype(np.float32)

    counts = np.bincount(ids, minlength=E)
    order = np.argsort(-counts, kind="stable")
    caps = tuple(int(counts[order[s * NCORES]]) for s in range(EPC))
    caps = tuple(max(c, 128) for c in caps)
    nloc = sum(caps)
    nc = _get_program(caps)

    tok = [np.where(ids == e)[0] for e in range(E)]

    # ---- per-core inputs ----
    key = tuple(id(inputs[k]) for k in
                ("routed_gate_w", "routed_up_w", "routed_down_w",
                 "shared_gate_w", "shared_up_w", "shared_down_w"))
    wcache = _PREP_CACHE.get("wkey") == key
    if not wcache:
        _PREP_CACHE.clear()
        _PREP_CACHE["wkey"] = key
        _PREP_CACHE["rgw"] = _to_bf16(np.asarray(inputs["routed_gate_w"], np.float32))
        _PREP_CACHE["ruw"] = _to_bf16(np.asarray(inputs["routed_up_w"], np.float32))
        _PREP_CACHE["rdw"] = _to_bf16(np.asarray(inputs["routed_down_w"], np.float32))
        _PREP_CACHE["sgw"] = _to_bf16(np.asarray(inputs["shared_gate_w"], np.float32))
        _PREP_CACHE["suw"] = _to_bf16(np.asarray(inputs["shared_up_w"], np.float32))
        _PREP_CACHE["sdw"] = _to_bf16(np.asarray(inputs["shared_down_w"], np.float32))
    rgw_all, ruw_all, rdw_all = (_PREP_CACHE["rgw"], _PREP_CACHE["ruw"],
                                 _PREP_CACHE["rdw"])
    sgw, suw, sdw = _PREP_CACHE["sgw"], _PREP_CACHE["suw"], _PREP_CACHE["sdw"]

    in_maps = []
    core_segs = []
    for c in range(NCORES):
        segs = [int(order[s * NCORES + c]) for s in range(EPC)]
        core_segs.append(segs)
        xs_loc = np.zeros((nloc, H), np.float32)
        off = 0
        for s, e in enumerate(segs):
            tl = tok[e]
            xs_loc[off:off + len(tl)] = x[tl] * scores[tl][:, None]
            off += caps[s]
        xsT_np = np.ascontiguousarray(xs_loc.T.reshape(HC, 128, nloc)).astype(BF16_NP)
        xshT_np = np.ascontiguousarray(
            x[c * TSH:(c + 1) * TSH].T.reshape(HC, 128, TSH)
        ).astype(BF16_NP)
        in_maps.append({
            "xsT": xsT_np,
            "xshT": xshT_np,
            "rgw": np.ascontiguousarray(rgw_all[segs]),
            "ruw": np.ascontiguousarray(ruw_all[segs]),
            "rdw": np.ascontiguousarray(rdw_all[segs]),
            "sgw": sgw, "suw": suw, "sdw": sdw,
        })

    res = run_bass_kernel_spmd(nc, in_maps, core_ids=list(range(NCORES)),
                               trace=TRACE)
    LAST_RESULTS = res

    # ---- combine ----
    out = np.zeros((T, H), np.float32)
    for c in range(NCORES):
        routT = res.results[c]["routT"].reshape(H, nloc)
        off = 0
        for s, e in enumerate(core_segs[c]):
            tl = tok[e]
            out[tl] = routT[:, off:off + len(tl)].T
            off += caps[s]
    for c in range(NCORES):
        shoutT = res.results[c]["shoutT"].reshape(H, TSH)
        out[c * TSH:(c + 1) * TSH] += shoutT.T
    return out.reshape(B, S, H)
